# revision 2
# baseline (speedup 1.0000x reference)
"""Trainium2 Bass kernel: factored-grid (triplane-style) embedding lookup + MLP.

Sharding: data-parallel over rays across 8 NeuronCores; grid patch-tables and
MLP weights replicated. Per core the device does 36 dma_gather calls per
ray-chunk (fp16 256B patch elements), DVE slot-weighting + reduction to
feats[rays, 288], PE transpose, K=288 matmul + relu(+b1), K=128 matmul +
sigmoid(+b2), emitting out [3, rays]. Host concatenates and transposes.
"""
import numpy as np
import ml_dtypes

import concourse.bacc as bacc
import concourse.bass as bass
import concourse.mybir as mybir
import concourse.tile as tile
from concourse.masks import make_identity
from concourse.bass_utils import run_bass_kernel_spmd

# ---- problem constants (hardcoded) ----
N_RAYS = 262144
COMBS = [(0, 1), (0, 2), (0, 3), (1, 2), (1, 3), (2, 3)]
LEVELS = [128, 256, 512]
N_CORES = 8
N_PER_CORE = N_RAYS // N_CORES           # 32768
E = 128                                  # fp16 values per element (256B)
CHUNK = 4096
JC = CHUNK // 128                        # 32
NCHUNK = N_PER_CORE // CHUNK             # 8

# calls = (level, comb, subtable); l2 planes split into 4 subtables
CALLS = []
for _li, _H in enumerate(LEVELS):
    for _ci in range(6):
        for _s in range(4 if _H == 512 else 1):
            CALLS.append((_li, _ci, _s))
N_CALLS = len(CALLS)                     # 36

_cache = {}


# ---------------- wait legalization (walrus: max 1 sync wait/inst) ---------
def _legalize_waits(nc):
    for f in nc.m.functions:
        for blk in f.blocks:
            bbs = blk.basic_blocks if hasattr(blk, "basic_blocks") else [blk]
            for bb in bbs:
                idx = 0
                while idx < len(bb.instructions):
                    inst = bb.instructions[idx]
                    si = inst.sync_info
                    if si is None:
                        idx += 1
                        continue
                    waits = list(si.on_wait)
                    if len(waits) <= 1:
                        idx += 1
                        continue
                    keep, excess = waits[:1], waits[1:]
                    for w in excess:
                        nop = mybir.InstNoOp(
                            name=nc.get_next_instruction_name(),
                            ins=[], outs=[])
                        nop.engine = inst.engine
                        nop.bass_nofuse = True
                        nop.sync_info = mybir.SyncInfo(
                            on_wait=[w], on_update=[])
                        nc.register_instruction(nop, overwrite=True)
                        bb.instructions.insert(idx, nop)
                        idx += 1
                    si.on_wait = keep
                    inst.sync_info = si
                    idx += 1


# ---------------- host-side table / index / weight prep -------------------
def _build_tables(grids):
    """grids: list of 3 arrays [6,16,H,W]. Element (u0, j=v0>>1) stores
    value[k, s], s = vh*4 + uc*2 + vc, = g[k, u0+uc, min(2j+vh+vc, W-1)].
    Returns fp16 megatable [R, 128] and per-plane meta."""
    tabs, meta, base = [], [], 0
    for g, H in zip(grids, LEVELS):
        g = np.asarray(g, np.float32)
        W = H
        nu, nj = H - 1, W // 2
        vh = np.arange(2)
        uc = np.arange(2)
        vc = np.arange(2)
        j = np.arange(nj)
        vidx = np.minimum(2 * j[:, None, None] + vh[None, :, None]
                          + vc[None, None, :], W - 1)          # [nj,2,2]
        for ci in range(6):
            gc = g[ci]                                          # [16,H,W]
            u0 = np.arange(nu)
            rowsel = gc[:, u0[:, None] + uc[None, :], :]        # [16,nu,2,W]
            t = rowsel[:, :, :, vidx]                           # [16,nu,2,nj,2,2]
            # -> [u0, j, k, vh, uc, vc]
            t = np.transpose(t, (1, 3, 0, 4, 2, 5))
            tabs.append(np.ascontiguousarray(
                t.reshape(nu * nj, 128)).astype(np.float16))
            meta.append((base, nu * nj, nu, nj))
            base += nu * nj
    return np.concatenate(tabs, axis=0), meta


def _subtables(meta):
    """Per call: (mega_base_row, sub_lo, sub_hi) in plane-row space."""
    out, pl = [], 0
    for li, H in enumerate(LEVELS):
        for ci in range(6):
            base, rows, nu, nj = meta[pl]
            nsub = 4 if H == 512 else 1
            sr = rows // nsub
            for s in range(nsub):
                lo = s * sr
                hi = rows if s == nsub - 1 else (s + 1) * sr
                out.append((base + lo, lo, hi))
            pl += 1
    return out


def _host_index_weights(ray, meta, subs):
    """idx int16 wrapped+replicated per (call, chunk) and fp16 slot weights."""
    n = ray.shape[0]
    idx_r = np.zeros((N_CALLS * NCHUNK, 128, CHUNK // 16), np.int16)
    w_r = np.zeros((NCHUNK, N_CALLS, 128, JC * 8), np.float16)
    call = 0
    pl = 0
    for li, H in enumerate(LEVELS):
        W = H
        for ci in range(6):
            a, b = COMBS[ci]
            base, rows, nu, nj = meta[pl]
            u = ray[:, a].astype(np.float64) * (H - 1)
            v = ray[:, b].astype(np.float64) * (W - 1)
            u0 = np.clip(np.floor(u), 0, H - 2).astype(np.int64)
            v0 = np.clip(np.floor(v), 0, W - 2).astype(np.int64)
            wu = (u - u0).astype(np.float32)
            wv = (v - v0).astype(np.float32)
            row = u0 * nj + (v0 >> 1)
            vh = (v0 & 1).astype(np.int64)
            wts = np.zeros((n, 8), np.float32)
            cu = np.stack([1 - wu, wu], 1)
            cv = np.stack([1 - wv, wv], 1)
            ar = np.arange(n)
            for uc in range(2):
                for vc in range(2):
                    wts[ar, vh * 4 + uc * 2 + vc] = cu[:, uc] * cv[:, vc]
            nsub = 4 if H == 512 else 1
            for s in range(nsub):
                _, lo, hi = subs[call]
                inr = (row >= lo) & (row < hi)
                loc = np.clip(row - lo, 0, hi - lo - 1).astype(np.int16)
                wrapped = loc.reshape(n // 16, 16).T            # [16, n/16]
                for chm in range(NCHUNK):
                    cw = CHUNK // 16
                    idx_r[call * NCHUNK + chm] = np.tile(
                        wrapped[:, chm * cw:(chm + 1) * cw], (8, 1))
                wmask = np.where(inr[:, None], wts, 0).astype(np.float16)
                for chm in range(NCHUNK):
                    blk = wmask[chm * CHUNK:(chm + 1) * CHUNK]  # [CHUNK,8]
                    w_r[chm, call] = blk.reshape(JC, 128, 8) \
                        .transpose(1, 0, 2).reshape(128, JC * 8)
                call += 1
            pl += 1
    return idx_r, w_r


# ---------------- device kernel -------------------------------------------
def _build_kernel(n_rows, subs, meta):
    # plane id per call + first-subtable flag
    plane_of = []
    pl = 0
    for li, H in enumerate(LEVELS):
        for ci in range(6):
            nsub = 4 if H == 512 else 1
            for s in range(nsub):
                plane_of.append((pl, s == 0))
            pl += 1

    nc = bacc.Bacc()
    mega = nc.dram_tensor("mega", [n_rows, E], mybir.dt.float16,
                          kind="ExternalInput")
    idxs = nc.dram_tensor("idxs", [N_CALLS * NCHUNK, 128, CHUNK // 16],
                          mybir.dt.int16, kind="ExternalInput")
    wts = nc.dram_tensor("wts", [NCHUNK, N_CALLS, 128, JC * 8],
                         mybir.dt.float16, kind="ExternalInput")
    w1x = nc.dram_tensor("w1x", [288, 128], mybir.dt.bfloat16,
                         kind="ExternalInput")
    b1x = nc.dram_tensor("b1x", [128, 1], mybir.dt.float32,
                         kind="ExternalInput")
    w2x = nc.dram_tensor("w2x", [128, 4], mybir.dt.bfloat16,
                         kind="ExternalInput")
    b2x = nc.dram_tensor("b2x", [4, 1], mybir.dt.float32,
                         kind="ExternalInput")
    out = nc.dram_tensor("out", [4, N_PER_CORE], mybir.dt.float32,
                         kind="ExternalOutput")

    with tile.TileContext(nc) as tc:
        with (
            tc.tile_pool(name="consts", bufs=1) as cp,
            tc.tile_pool(name="gather", bufs=2) as gp,
            tc.tile_pool(name="feats", bufs=2) as fpool,
            tc.tile_pool(name="psum", bufs=4, space="PSUM") as pp,
            tc.tile_pool(name="psum2", bufs=2, space="PSUM") as pp2,
        ):
            ident = cp.tile([128, 128], mybir.dt.bfloat16)
            make_identity(nc, ident[:])
            w1t = cp.tile([128, 3, 128], mybir.dt.bfloat16)
            nc.vector.memset(w1t[:], 0.0)
            for kk in range(3):
                rows = 128 if kk < 2 else 32
                nc.sync.dma_start(out=w1t[:rows, kk, :],
                                  in_=w1x[kk * 128:kk * 128 + rows, :])
            b1t = cp.tile([128, 1], mybir.dt.float32)
            nc.sync.dma_start(out=b1t[:], in_=b1x[:])
            w2t = cp.tile([128, 4], mybir.dt.bfloat16)
            nc.sync.dma_start(out=w2t[:], in_=w2x[:])
            b2t = cp.tile([4, 1], mybir.dt.float32)
            nc.sync.dma_start(out=b2t[:], in_=b2x[:])

            for ch in range(NCHUNK):
                feats = fpool.tile([128, JC, 304], mybir.dt.bfloat16,
                                   tag="feats")
                nc.vector.memset(feats[:, :, 288:304], 0.0)
                for c in range(N_CALLS):
                    pl, first = plane_of[c]
                    mbase, lo, hi = subs[c]
                    idx_t = gp.tile([128, CHUNK // 16], mybir.dt.int16,
                                    tag="idx")
                    nc.sync.dma_start(out=idx_t[:],
                                      in_=idxs[c * NCHUNK + ch, :, :])
                    wt_t = gp.tile([128, JC, 1, 8], mybir.dt.float16,
                                   tag="wt")
                    nc.sync.dma_start(
                        out=wt_t[:],
                        in_=wts[ch, c, :, :].rearrange(
                            "p (j o s) -> p j o s", o=1, s=8))
                    patch = gp.tile([128, JC, E], mybir.dt.float16,
                                    tag="patch")
                    nc.gpsimd.dma_gather(
                        out_ap=patch[:],
                        in_ap=mega[mbase:mbase + (hi - lo), :],
                        idxs_ap=idx_t[:],
                        num_idxs=CHUNK,
                        num_idxs_reg=CHUNK,
                        elem_size=E,
                        single_packet=False,
                    )
                    wgt = gp.tile([128, JC, 16, 8], mybir.dt.bfloat16,
                                  tag="wgt")
                    nc.vector.tensor_tensor(
                        out=wgt[:],
                        in0=patch[:].rearrange("p j (k s) -> p j k s", s=8),
                        in1=wt_t[:].to_broadcast([128, JC, 16, 8]),
                        op=mybir.AluOpType.mult,
                    )
                    r4 = gp.tile([128, JC, 16, 4], mybir.dt.bfloat16,
                                 tag="r4")
                    nc.vector.tensor_tensor(
                        out=r4[:], in0=wgt[:, :, :, 0:4],
                        in1=wgt[:, :, :, 4:8], op=mybir.AluOpType.add)
                    r2 = gp.tile([128, JC, 16, 2], mybir.dt.bfloat16,
                                 tag="r2")
                    nc.vector.tensor_tensor(
                        out=r2[:], in0=r4[:, :, :, 0:2],
                        in1=r4[:, :, :, 2:4], op=mybir.AluOpType.add)
                    dst = feats[:, :, pl * 16:(pl + 1) * 16]
                    if first:
                        nc.vector.tensor_tensor(
                            out=dst, in0=r2[:, :, :, 0], in1=r2[:, :, :, 1],
                            op=mybir.AluOpType.add)
                    else:
                        r1 = gp.tile([128, JC, 16], mybir.dt.bfloat16,
                                     tag="r1")
                        nc.vector.tensor_tensor(
                            out=r1[:], in0=r2[:, :, :, 0], in1=r2[:, :, :, 1],
                            op=mybir.AluOpType.add)
                        nc.vector.tensor_tensor(
                            out=dst, in0=dst, in1=r1[:],
                            op=mybir.AluOpType.add)

                ftT = fpool.tile([128, 3, CHUNK], mybir.dt.bfloat16,
                                 tag="ftT")
                for j in range(JC):
                    for kk in range(3):
                        rows = 128 if kk < 2 else 48
                        tp = pp.tile([128, 128], mybir.dt.bfloat16, tag="tp")
                        nc.tensor.transpose(
                            out=tp[:rows, :],
                            in_=feats[:, j, kk * 128:kk * 128 + rows],
                            identity=ident[:],
                        )
                        nc.vector.tensor_copy(
                            out=ftT[:rows, kk, j * 128:(j + 1) * 128],
                            in_=tp[:rows, :])

                hT = fpool.tile([128, CHUNK], mybir.dt.bfloat16, tag="hT")
                oT = fpool.tile([4, CHUNK], mybir.dt.float32, tag="oT")
                for q in range(CHUNK // 512):
                    hp = pp2.tile([128, 512], mybir.dt.float32, tag="hp")
                    for kk in range(3):
                        rows = 128 if kk < 2 else 32
                        nc.tensor.matmul(
                            out=hp[:],
                            lhsT=w1t[:rows, kk, :],
                            rhs=ftT[:rows, kk, q * 512:(q + 1) * 512],
                            start=(kk == 0),
                            stop=(kk == 2),
                        )
                    nc.scalar.activation(
                        out=hT[:, q * 512:(q + 1) * 512], in_=hp[:],
                        func=mybir.ActivationFunctionType.Relu,
                        bias=b1t[:],
                    )
                    op_ = pp2.tile([4, 512], mybir.dt.float32, tag="op")
                    nc.tensor.matmul(
                        out=op_[:], lhsT=w2t[:, :],
                        rhs=hT[:, q * 512:(q + 1) * 512],
                        start=True, stop=True,
                    )
                    nc.scalar.activation(
                        out=oT[:, q * 512:(q + 1) * 512], in_=op_[:],
                        func=mybir.ActivationFunctionType.Sigmoid,
                        bias=b2t[:],
                    )
                nc.sync.dma_start(out=out[:, ch * CHUNK:(ch + 1) * CHUNK],
                                  in_=oT[:])
    nc.compile()
    _legalize_waits(nc)
    return nc


# ---------------- entry point ---------------------------------------------
def kernel(ray, grids_l0, grids_l1, grids_l2, w1, b1, w2, b2):
    mega, meta = _build_tables([grids_l0, grids_l1, grids_l2])
    subs = _subtables(meta)
    if "nc" not in _cache:
        _cache["nc"] = _build_kernel(mega.shape[0], subs, meta)
    nc = _cache["nc"]

    w1b = np.asarray(w1, np.float32).astype(ml_dtypes.bfloat16)
    w2b = np.zeros((128, 4), ml_dtypes.bfloat16)
    w2b[:, :3] = np.asarray(w2, np.float32).astype(ml_dtypes.bfloat16)
    b1c = np.asarray(b1, np.float32).reshape(128, 1)
    b2c = np.zeros((4, 1), np.float32)
    b2c[:3, 0] = np.asarray(b2, np.float32)

    ray = np.asarray(ray, np.float32)
    in_maps = []
    for core in range(N_CORES):
        sl = ray[core * N_PER_CORE:(core + 1) * N_PER_CORE]
        idx_r, w_r = _host_index_weights(sl, meta, subs)
        in_maps.append({
            "mega": mega, "idxs": idx_r, "wts": w_r,
            "w1x": w1b, "b1x": b1c, "w2x": w2b, "b2x": b2c,
        })

    res = run_bass_kernel_spmd(nc, in_maps, list(range(N_CORES)))
    _cache["last_res"] = res
    outs = [np.ascontiguousarray(res.results[c]["out"][:3].T)
            for c in range(N_CORES)]
    return np.concatenate(outs, axis=0).astype(np.float32)

